# revision 10
# baseline (speedup 1.0000x reference)
"""Conv2D 3x3 stride-1 pad-1 (NCHW) via 1D Winograd F(2,3) along H, on 8
NeuronCores (data-parallel over batch, 4 images per core).

Direct implicit-GEMM needs 9 taps x 2 out-chunks = 18 matmul-columns per
output position (PE floor ~95us/core). F(2,3) along H replaces the 3 kh
taps with 4 Winograd points at half the output rows each: 4 j x 3 kw x 2
oc = 12 columns per position -> PE floor ~66us/core.

Per image: DVE computes V_j from padded bf16 rows (d0..d3 = rows 2i..2i+3):
  V1=d0-d2, V2=d1+d2, V3=d2-d1, V4=d1-d3      (4 tensor_tensor ops, 2x bf16)
PE contracts cin(128) x kw(3) per (j, oc) into 4 PSUM banks per group of
7 row-pairs (N=392, 166ns warm cadence). Host-transformed weights:
w~1=f0, w~2=(f0+f1+f2)/2, w~3=(f0-f1+f2)/2, w~4=f2.  ScalarE evacuates
all four m_j PSUM banks to SBUF bf16 (ACTIVATE copy ~370ns); DVE does the
output transform y_even=m1+m2+m3, y_odd=m2-m3-m4 at image-level FD=1568
in 2x bf16 mode. Output stored bf16; host converts to f32 and adds bias
(bias is zero in this problem family anyway).

Head: junk matmuls gated on a gpsimd memset warm the PE clock (HAM) from
~7.7us; weights arrive as 4 per-j DMA slices and image 0 in 3 row chunks
with matching V-transform chunks, and the first group runs j-major across
both oc chunks so weight demand matches ring delivery (~1us/slice). Last
image-oc uses groups (7,7,7,5,2) and segmented y stores for a short tail.
"""

import sys

import numpy as np

if "/opt/trn_rl_repo" not in sys.path:
    sys.path.insert(0, "/opt/trn_rl_repo")

from concourse import bacc, bass, mybir  # noqa: E402
from concourse.bass_utils import run_bass_kernel_spmd  # noqa: E402
from concourse.tile import TileContext  # noqa: E402

N_FULL, CIN, H, W = 32, 128, 56, 56
COUT = 256
KW = 3
NJ = 4  # winograd points
NCORES = 8
NPER = N_FULL // NCORES  # 4 images per core
HP, WP = H + 2, W + 2  # 58 x 58 padded
NPAIR = H // 2  # 28 output row-pairs
RP = 7  # row-pairs per matmul group
NFREE = RP * W  # 392 moving free dim
OCH = COUT // 128  # 2 output-channel chunks

_CACHE = {}


def _build_conv():
    f32 = mybir.dt.float32
    bf16 = mybir.dt.float16  # fp16: same PE/DVE speed class as bf16, 8x finer mantissa

    nc = bacc.Bacc(None, target_bir_lowering=False)

    x_par = nc.declare_dram_parameter("x", [NPER, CIN, HP, WP], bf16, isOutput=False)
    w_par = nc.declare_dram_parameter(
        "wt", [CIN, NJ * KW * COUT], bf16, isOutput=False
    )
    out_par = nc.declare_dram_parameter("out", [NPER, COUT, H, W], bf16, isOutput=True)
    out_flat = out_par.rearrange("n o h w -> n o (h w)")

    with TileContext(nc) as tc:
        with (
            tc.tile_pool(name="const", bufs=1) as cpool,
            tc.tile_pool(name="psum", bufs=8, space="PSUM") as ppool,
            tc.tile_pool(name="mc", bufs=3) as mpool,
            tc.tile_pool(name="tsc", bufs=4) as tpool,
            tc.tile_pool(name="outp", bufs=3) as opool,
        ):
            # --- HAM pre-warm: junk matmuls gated only on a gpsimd memset,
            # so the PE clock gate reaches 8/8 before the real stream.
            jnk = cpool.tile([128, 512], f32, tag="jnk")
            nc.gpsimd.memset(jnk[:], 1.0)
            jnk_mm = jnk.bitcast(bf16)
            ps_jnk = ppool.tile([128, NFREE], f32, tag="ps", name="ps")
            for _ in range(12):
                nc.tensor.matmul(
                    ps_jnk[:],
                    jnk_mm[:, 0:128],
                    jnk_mm[:, 0:NFREE],
                    start=True,
                    stop=True,
                )
            # gpsimd tensor_tensor probe ops (throughput measurement for a
            # possible F(4,3) upgrade); results unused.
            scr = cpool.tile([128, 812], bf16, tag="scr")
            for _ in range(2):
                nc.gpsimd.tensor_sub(scr[:], jnk_mm[:, 0:812], jnk_mm[:, 0:812])

            x_sb = cpool.tile([CIN, NPER, HP, WP], bf16, tag="xall", name="xall")
            w_sb = cpool.tile([CIN, NJ * KW * COUT], bf16, tag="w", name="w")
            # V_j transforms, all four images resident: [CIN, n, j, 28, 58]
            v_sb = cpool.tile([CIN, NPER, NJ, NPAIR, WP], bf16, tag="v", name="v")

            w3_sb = w_sb.rearrange("p (j k o) -> p j k o", j=NJ, k=KW)
            w3_dr = w_par[:].rearrange("p (j k o) -> p j k o", j=NJ, k=KW)
            # padded rows split as (pair, parity) for stride-2 row access
            x3 = x_sb.rearrange("p n (hp two) w -> p n hp two w", two=2)

            # --- input DMAs.  scalar ring: weights (one slice per j, in
            # consumption order) then x2,x3.  sync ring: x0 in 3 chunks
            # matched to V-transform chunks, then x1.
            nc.sync.dma_start(out=x_sb[:, 0, 0:16, :], in_=x_par[0, :, 0:16, :])
            nc.scalar.dma_start(out=w3_sb[:, 0], in_=w3_dr[:, 0])
            nc.scalar.dma_start(out=w3_sb[:, 1], in_=w3_dr[:, 1])
            nc.sync.dma_start(out=x_sb[:, 0, 16:30, :], in_=x_par[0, :, 16:30, :])
            nc.scalar.dma_start(out=w3_sb[:, 2], in_=w3_dr[:, 2])
            nc.sync.dma_start(out=w3_sb[:, 3], in_=w3_dr[:, 3])
            nc.scalar.dma_start(out=x_sb[:, 0, 30:58, :], in_=x_par[0, :, 30:58, :])
            nc.sync.dma_start(out=x_sb[:, 1, :, :], in_=x_par[1])
            nc.scalar.dma_start(out=x_sb[:, 2, :, :], in_=x_par[2])
            nc.scalar.dma_start(out=x_sb[:, 3, :, :], in_=x_par[3])

            def emit_v(n, p0, p1):
                """V_j for image n, row-pairs [p0, p1) on DVE."""
                d0 = x3[:, n, p0:p1, 0, :]
                d1 = x3[:, n, p0:p1, 1, :]
                d2 = x3[:, n, p0 + 1 : p1 + 1, 0, :]
                d3 = x3[:, n, p0 + 1 : p1 + 1, 1, :]
                nc.vector.tensor_sub(v_sb[:, n, 0, p0:p1, :], d0, d2)
                nc.vector.tensor_add(v_sb[:, n, 1, p0:p1, :], d1, d2)
                nc.vector.tensor_sub(v_sb[:, n, 2, p0:p1, :], d2, d1)
                nc.vector.tensor_sub(v_sb[:, n, 3, p0:p1, :], d1, d3)

            # image 0 transform in three chunks tracking its DMA chunks
            emit_v(0, 0, 7)
            emit_v(0, 7, 14)
            emit_v(0, 14, NPAIR)

            # groups of row-pairs per (img, oc): uniform 7s, except the very
            # last img-oc which tapers (7,7,7,5,2) for a short tail.
            full_groups = [(g * RP, RP) for g in range(NPAIR // RP)]
            tail_groups = full_groups[:-1] + [(21, 5), (26, 2)]
            # y-assembly segments (pair ranges) per img-oc; the final (26,28)
            # group of the last img-oc is assembled straight from PSUM
            full_segs = [(0, NPAIR)]
            tail_segs = [(0, 14), (14, 26)]

            store_idx = 0

            def assemble_and_store(mc, n, oc, p0, p1, last):
                """Output transform for pairs [p0,p1) + store (DVE + DMA)."""
                nonlocal store_idx
                npair = p1 - p0
                t1 = tpool.tile([128, NPAIR * W], bf16, tag="t", name="t")
                t2 = tpool.tile([128, NPAIR * W], bf16, tag="t", name="t")
                y = opool.tile([128, NPAIR, 2, W], bf16, tag="y", name="y")
                nfree = npair * W
                m1 = mc[:, 0, p0:p1, :]
                m2 = mc[:, 1, p0:p1, :]
                m3 = mc[:, 2, p0:p1, :]
                m4 = mc[:, 3, p0:p1, :]
                t1v = t1[:, 0:nfree].rearrange("p (r w) -> p r w", w=W)
                t2v = t2[:, 0:nfree].rearrange("p (r w) -> p r w", w=W)
                nc.vector.tensor_add(t1v, m1, m2)
                nc.vector.tensor_add(y[:, p0:p1, 0, :], t1v, m3)
                nc.vector.tensor_sub(t2v, m2, m3)
                nc.vector.tensor_sub(y[:, p0:p1, 1, :], t2v, m4)
                dst = out_flat[
                    n, oc * 128 : (oc + 1) * 128, p0 * 2 * W : p1 * 2 * W
                ]
                src = y[:, p0:p1].rearrange("p r two w -> p (r two w)")
                if last:
                    half = nfree  # = half of the 2*nfree output elements
                    nc.sync.dma_start(out=dst[:, 0:half], in_=src[:, 0:half])
                    nc.scalar.dma_start(
                        out=dst[:, half : 2 * nfree], in_=src[:, half : 2 * nfree]
                    )
                else:
                    nc.sync.dma_start(out=dst, in_=src)
                store_idx += 1

            def emit_group_mms(ps, n, oc, p0, npr, j):
                for kw in range(KW):
                    nc.tensor.matmul(
                        ps[:],
                        w3_sb[:, j, kw, oc * 128 : oc * 128 + 128],
                        v_sb[:, n, j, p0 : p0 + npr, kw : kw + W],
                        start=(kw == 0),
                        stop=(kw == KW - 1),
                    )

            # --- first group of image 0: j-major across BOTH oc chunks so
            # the weight demand rate (~1 j-slice/us) matches ring delivery.
            mc_hold = [
                mpool.tile([128, NJ, NPAIR, W], bf16, tag="mc", name="mc")
                for _ in range(OCH)
            ]
            ps_g0 = {}
            for j in range(NJ):
                for oc in range(OCH):
                    ps = ppool.tile([128, RP * W], f32, tag="ps", name="ps")
                    ps_g0[(oc, j)] = ps
                    emit_group_mms(ps, 0, oc, 0, RP, j)
            for j in range(NJ):
                for oc in range(OCH):
                    nc.scalar.copy(
                        mc_hold[oc][:, j, 0:RP, :], ps_g0[(oc, j)][:]
                    )

            for n in range(NPER):
                for oc in range(OCH):
                    is_tail = n == NPER - 1 and oc == OCH - 1
                    groups = tail_groups if is_tail else full_groups
                    segs = tail_segs if is_tail else full_segs
                    if n == 0:
                        mc = mc_hold[oc]
                    else:
                        mc = mpool.tile(
                            [128, NJ, NPAIR, W], bf16, tag="mc", name="mc"
                        )
                    seg_i = 0
                    for gi, (p0, npr) in enumerate(groups):
                        if n == 0 and gi == 0:
                            continue  # emitted above
                        ps = [
                            ppool.tile([128, npr * W], f32, tag="ps", name="ps")
                            for _ in range(NJ)
                        ]
                        for j in range(NJ):
                            emit_group_mms(ps[j], n, oc, p0, npr, j)
                        if is_tail and gi == len(groups) - 1:
                            # final 2-pair group: y straight from PSUM on DVE
                            # (skips the copy stage to shorten the drain tail)
                            nfree = npr * W
                            t1 = tpool.tile([128, NPAIR * W], bf16, tag="t")
                            t2 = tpool.tile([128, NPAIR * W], bf16, tag="t")
                            y = opool.tile([128, NPAIR, 2, W], bf16, tag="y")
                            t1v = t1[:, 0:nfree].rearrange(
                                "p (r w) -> p r w", w=W
                            )
                            t2v = t2[:, 0:nfree].rearrange(
                                "p (r w) -> p r w", w=W
                            )
                            psv = [
                                p[:].rearrange("p (r w) -> p r w", w=W)
                                for p in ps
                            ]
                            # tensor_tensor allows at most one PSUM input:
                            # stage m2/m3 via quick ScalarE copies first
                            m2s = tpool.tile([128, NPAIR * W], bf16, tag="t")
                            m3s = tpool.tile([128, NPAIR * W], bf16, tag="t")
                            m2v = m2s[:, 0:nfree].rearrange(
                                "p (r w) -> p r w", w=W
                            )
                            m3v = m3s[:, 0:nfree].rearrange(
                                "p (r w) -> p r w", w=W
                            )
                            nc.scalar.copy(m2v, psv[1])
                            nc.scalar.copy(m3v, psv[2])
                            nc.vector.tensor_add(t1v, m2v, psv[0])
                            nc.vector.tensor_add(
                                y[:, p0 : p0 + npr, 0, :], t1v, psv[2]
                            )
                            nc.vector.tensor_sub(t2v, m2v, m3v)
                            nc.vector.tensor_sub(
                                y[:, p0 : p0 + npr, 1, :], t2v, psv[3]
                            )
                            dst = out_flat[
                                n,
                                oc * 128 : (oc + 1) * 128,
                                p0 * 2 * W : (p0 + npr) * 2 * W,
                            ]
                            src = y[:, p0 : p0 + npr].rearrange(
                                "p r two w -> p (r two w)"
                            )
                            nc.sync.dma_start(
                                out=dst[:, 0:nfree], in_=src[:, 0:nfree]
                            )
                            nc.scalar.dma_start(
                                out=dst[:, nfree : 2 * nfree],
                                in_=src[:, nfree : 2 * nfree],
                            )
                            continue
                        for j in range(NJ):
                            nc.scalar.copy(mc[:, j, p0 : p0 + npr, :], ps[j][:])
                        pairs_done = p0 + npr
                        # V for the next image slots into DVE's queue here
                        if oc == 1 and gi == 1 and n + 1 < NPER:
                            emit_v(n + 1, 0, NPAIR)
                        # flush any y segments fully covered by copies so far
                        while seg_i < len(segs) and segs[seg_i][1] <= pairs_done:
                            s0, s1 = segs[seg_i]
                            assemble_and_store(
                                mc, n, oc, s0, s1,
                                last=is_tail and seg_i == len(segs) - 1,
                            )
                            seg_i += 1
    nc.compile()
    return nc


def _get_nc():
    if "nc" not in _CACHE:
        _CACHE["nc"] = _build_conv()
    return _CACHE["nc"]


# test-harness hooks: set TRACE=True before calling kernel() to capture an
# NTFF profile; LAST_RESULTS then holds the BassKernelResults.
TRACE = False
LAST_RESULTS = None


def kernel(x, weight, bias):
    global LAST_RESULTS
    bfl = np.float16
    x = np.ascontiguousarray(np.asarray(x), dtype=np.float32)
    w = np.ascontiguousarray(np.asarray(weight), dtype=np.float32)
    b = np.ascontiguousarray(np.asarray(bias), dtype=np.float32)
    xp = np.pad(x, ((0, 0), (0, 0), (1, 1), (1, 1))).astype(bfl)
    # winograd F(2,3) weight transform along kh; layout wt[i, (j kw o)]
    f0, f1, f2 = w[:, :, 0, :], w[:, :, 1, :], w[:, :, 2, :]
    w4 = np.stack(
        [f0, (f0 + f1 + f2) * 0.5, (f0 - f1 + f2) * 0.5, f2], axis=0
    )  # [4, O, I, KW]
    wt = np.ascontiguousarray(
        w4.transpose(2, 0, 3, 1).reshape(CIN, NJ * KW * COUT)
    ).astype(bfl)

    per_core = [
        {"x": xp[c * NPER : (c + 1) * NPER], "wt": wt}
        for c in range(NCORES)
    ]

    kwargs = {}
    if TRACE:
        kwargs = dict(trace=True, trace_cores=[0])
    res = run_bass_kernel_spmd(
        _get_nc(), per_core, core_ids=list(range(NCORES)), **kwargs
    )
    LAST_RESULTS = res
    out = np.concatenate([r["out"] for r in res.results], axis=0)
    out = out.astype(np.float32)
    if np.any(b):
        out += b[None, :, None, None]
    return out
